# revision 5
# baseline (speedup 1.0000x reference)
"""Trainium2 Bass kernel v3 for the 3-layer GAT denoising model.

Changes vs v2:
- Table split into two half-tables (separate DRAM tensors, rank-major
  AllGather each): T_A = dest tiles 0..24, T_B = tiles 25..48. int16 gather
  windows fall out naturally (25664 / 24640 rows).
- Edge phase split into an A-source pass over all dest tiles (buffering
  per-tile partial sums + denominators in SBUF) and a B-source pass that
  finalizes. Layer l+1's AG-A fires mid-B-pass (after dest tiles 0..24
  finalize), AG-B at the end -> collectives overlap edge compute instead of
  serializing with it.
- alpha_dst captured to SBUF at dense time (double-buffered), no DRAM
  roundtrip.
"""

import math
import os
import numpy as np

os.environ.setdefault("NEURON_RT_RESET_CORES", "1")

import concourse.bacc as bacc
import concourse.mybir as mybir
import concourse.tile as tile
from concourse.masks import make_identity

N_CORES = 8
TW = 384          # table row width (bf16 elems) = 768B
HC = 256
NH = 8
NT = 49           # dest tiles per core
TA_T = 25         # dest tiles in half A
TB_T = NT - TA_T  # 24
LA = TA_T * 128 + 8   # per-core rows in AGIN_A (incl 8 dummy)
LB = TB_T * 128 + 8
L = NT * 128          # real rows per core
F32 = mybir.dt.float32
BF = mybir.dt.float16     # 2-byte DVE fast path; fp16 mantissa >> bf16 here
I16 = mybir.dt.int16
AF = mybir.ActivationFunctionType
OP = mybir.AluOpType


# ----------------------------------------------------------------------------
# host preprocessing
# ----------------------------------------------------------------------------
def preprocess(adj, n):
    CAP_A = N_CORES * TA_T * 128   # 25600
    CAP_B = N_CORES * TB_T * 128   # 24576
    src = np.concatenate([adj[0], np.arange(n)]).astype(np.int64)
    dst = np.concatenate([adj[1], np.arange(n)]).astype(np.int64)

    deg = np.bincount(dst, minlength=n)
    order_tot = np.argsort(deg, kind="stable")
    # alternate assignment by degree order; cap half B at its capacity
    half_bit = np.zeros(n, dtype=bool)     # True -> B
    want_b = order_tot[1::2]
    nb = min(len(want_b), CAP_B)
    half_bit[want_b[:nb]] = True
    # (if B overflows capacity the rest stay in A; A capacity is checked below)
    src_is_b = half_bit[src]
    degA = np.bincount(dst[~src_is_b], minlength=n)
    degB = np.bincount(dst[src_is_b], minlength=n)

    A_nodes = np.flatnonzero(~half_bit)
    B_nodes = np.flatnonzero(half_bit)
    assert len(A_nodes) <= CAP_A and len(B_nodes) <= CAP_B
    dmax = np.maximum(degA, degB)
    dmin = np.minimum(degA, degB)
    A_sorted = A_nodes[np.lexsort((dmin[A_nodes], dmax[A_nodes]))]
    B_sorted = B_nodes[np.lexsort((dmin[B_nodes], dmax[B_nodes]))]
    A_list = np.concatenate([np.full(CAP_A - len(A_sorted), -1, np.int64), A_sorted])
    B_list = np.concatenate([np.full(CAP_B - len(B_sorted), -1, np.int64), B_sorted])

    # slot (half, group g, p): group g -> tile t = g//8 (within half), core k = g%8
    # phys row in half table: k*Lh + t*128 + p
    # local AGIN row: t*128 + p
    node2phys = np.full(n, -1, dtype=np.int64)   # encoded: half*2^20 + half-row
    node_core = np.full(n, -1, dtype=np.int64)
    node_tile = np.full(n, -1, dtype=np.int64)   # global dest tile 0..48
    node_p = np.full(n, -1, dtype=np.int64)

    for hb, lst, cap_t, Lh, t_off in ((0, A_list, TA_T, LA, 0),
                                      (1, B_list, TB_T, LB, TA_T)):
        g = np.arange(len(lst)) // 128
        p = np.arange(len(lst)) % 128
        t = g // 8
        k = g % 8
        real = lst >= 0
        nodes = lst[real]
        node2phys[nodes] = (hb << 20) | (k[real] * Lh + t[real] * 128 + p[real])
        node_core[nodes] = k[real]
        node_tile[nodes] = t[real] + t_off
        node_p[nodes] = p[real]
    assert (node2phys >= 0).all()

    # per (core, dest tile) ELL widths for A-src and B-src edges
    dk = node_core[dst]
    di = node_tile[dst]
    dp = node_p[dst]

    a_cnt = np.zeros(n, np.int64)
    b_cnt = np.zeros(n, np.int64)
    np.add.at(a_cnt, dst[~src_is_b], 1)
    np.add.at(b_cnt, dst[src_is_b], 1)
    DA = np.zeros((N_CORES, NT), np.int64)
    DB = np.zeros((N_CORES, NT), np.int64)
    np.maximum.at(DA, (node_core[node_core >= 0], node_tile[node_core >= 0]),
                  a_cnt[node_core >= 0])
    np.maximum.at(DB, (node_core[node_core >= 0], node_tile[node_core >= 0]),
                  b_cnt[node_core >= 0])
    DAi = np.maximum(DA.max(axis=0), 1)
    DBi = np.maximum(DB.max(axis=0), 1)

    coreA = [[np.full(128 * DAi[ii], LA - 8, np.int32) for ii in range(NT)]
             for _ in range(N_CORES)]
    coreB = [[np.full(128 * DBi[ii], LB - 8, np.int32) for ii in range(NT)]
             for _ in range(N_CORES)]

    es = np.lexsort((src, dst))
    ds_, isb_ = dst[es], src_is_b[es]
    dk_, di_, dp_ = dk[es], di[es], dp[es]
    sphys_ = node2phys[src[es]] & 0xFFFFF
    keys = ds_ * 2 + isb_.astype(np.int64)
    sort2 = np.argsort(keys, kind="stable")
    ks = keys[sort2]
    starts = np.r_[0, np.flatnonzero(np.diff(ks)) + 1]
    cum = np.arange(len(ks))
    seg_start = np.repeat(cum[starts], np.diff(np.r_[starts, len(ks)]))
    rank = cum - seg_start
    jcol = np.empty(len(ks), np.int64)
    jcol[sort2] = rank
    sizesA = 128 * DAi
    sizesB = 128 * DBi
    offA = np.concatenate([[0], np.cumsum(sizesA)[:-1]])
    offB = np.concatenate([[0], np.cumsum(sizesB)[:-1]])
    bigA = [np.concatenate(coreA[kk]) for kk in range(N_CORES)]
    bigB = [np.concatenate(coreB[kk]) for kk in range(N_CORES)]
    selB = isb_
    for kk in range(N_CORES):
        mA = (~selB) & (dk_ == kk)
        bigA[kk][offA[di_[mA]] + jcol[mA] * 128 + dp_[mA]] = sphys_[mA]
        mB = selB & (dk_ == kk)
        bigB[kk][offB[di_[mB]] + jcol[mB] * 128 + dp_[mB]] = sphys_[mB]
    for kk in range(N_CORES):
        for ii in range(NT):
            coreA[kk][ii] = bigA[kk][offA[ii]:offA[ii] + sizesA[ii]]
            coreB[kk][ii] = bigB[kk][offB[ii]:offB[ii] + sizesB[ii]]

    return dict(node_core=node_core, node_tile=node_tile, node_p=node_p,
                DAi=DAi, DBi=DBi, coreA=coreA, coreB=coreB, n=n)


def build_chunks(prep, cmax):
    """Per source-half groups over dest tiles: [(half, [(tile, cc), ...])].
    Adjacent dest tiles' blocks share one gather when widths fit GCAP."""
    DAi, DBi = prep["DAi"], prep["DBi"]
    for ii in range(NT):
        assert DAi[ii] <= cmax and DBi[ii] <= cmax
    GCAP = 24
    out = {"A": [], "B": []}
    for half, D in (("A", DAi), ("B", DBi)):
        i = 0
        while i < NT:
            if i + 1 < NT and D[i] + D[i + 1] <= GCAP:
                tiles = [i, i + 1]
            else:
                tiles = [i]
            out[half].append((half, [(t, int(D[t])) for t in tiles]))
            i += len(tiles)
    return out


def wrap_idx(block_i32):
    num = block_i32.shape[0]
    assert num % 16 == 0
    g = block_i32.reshape(num // 16, 16).T.astype(np.int16)
    return np.tile(g, (8, 1))  # [128, num/16]


# interleave: new col j = c*8+h maps to old col h*32+c
_ih = np.arange(HC)
PERM_OLD_FOR_NEW = (_ih % NH) * 32 + (_ih // NH)   # newcol j <- old PERM[j]


def host_inputs(inputs, prep, chunks):
    n = prep["n"]
    x = np.asarray(inputs["x"], np.float32)
    qY = np.asarray(inputs["q_Y_sample"], np.float32)
    NF = x.shape[1]

    # per-core x/qY in local row order (tile-major, 49*128 rows)
    node_core, node_tile, node_p = (prep["node_core"], prep["node_tile"],
                                    prep["node_p"])
    xs = np.zeros((N_CORES, L, NF), np.float32)
    qYs = np.zeros((N_CORES, L, NH), np.float32)
    rows = node_tile * 128 + node_p
    xs[node_core, rows] = x
    qYs[node_core, rows] = qY

    W = [np.asarray(inputs[f"W{i}"], np.float32) for i in range(3)]
    att_src = np.asarray(inputs["att_src"], np.float32)
    att_dst = np.asarray(inputs["att_dst"], np.float32)
    bias = np.asarray(inputs["bias"], np.float32)
    Whats = []
    for l in range(3):
        As = np.zeros((HC, NH), np.float32)
        Ad = np.zeros((HC, NH), np.float32)
        for hh in range(NH):
            As[hh * 32:(hh + 1) * 32, hh] = att_src[l, hh]
            Ad[hh * 32:(hh + 1) * 32, hh] = att_dst[l, hh]
        Wl = W[l]
        if l > 0:
            Wl = np.concatenate([Wl[:HC][PERM_OLD_FOR_NEW], Wl[HC:]], axis=0)
        Wh = np.zeros((Wl.shape[0], TW), np.float32)
        Wh[:, :HC] = Wl[:, PERM_OLD_FOR_NEW]
        Wh[:, HC:HC + NH] = Wl @ As
        Wh[:, HC + NH:HC + 2 * NH] = Wl @ Ad
        Whats.append(Wh.astype(np.float32))

    half = 64
    freqs4 = np.exp(np.arange(half, dtype=np.float32)
                    * (-math.log(10000.0) / (half - 1))).astype(np.float32)
    b_rep = np.stack([np.tile(bias[l][PERM_OLD_FOR_NEW][None, :], (128, 1))
                      for l in range(3)])

    fin_w1 = np.asarray(inputs["fin_w1"], np.float32)
    fin_w1 = np.concatenate([fin_w1[:HC][PERM_OLD_FOR_NEW], fin_w1[HC:]], axis=0)
    fin_b1 = np.asarray(inputs["fin_b1"], np.float32)
    fin_w2 = np.asarray(inputs["fin_w2"], np.float32)
    fin_b2 = np.asarray(inputs["fin_b2"], np.float32)

    tmlp_w2 = np.asarray(inputs["tmlp_w2"], np.float32)[:, PERM_OLD_FOR_NEW]
    tmlp_b2 = np.asarray(inputs["tmlp_b2"], np.float32)[PERM_OLD_FOR_NEW]

    dummy = np.zeros((8, TW), np.float32)
    dummy[:, HC:HC + NH] = -30000.0

    common = {
        "What0": Whats[0].astype(np.float32), "What1": Whats[1], "What2": Whats[2],
        "b_rep": b_rep.astype(np.float32),
        "fin_w1": fin_w1, "fin_b1rep": np.tile(fin_b1[None, :], (128, 1)).astype(np.float32),
        "fin_w2": fin_w2, "fin_b2rep": np.tile(fin_b2[None, :], (128, 1)).astype(np.float32),
        "tmlp_w1": np.asarray(inputs["tmlp_w1"], np.float32),
        "tmlp_b1col": np.asarray(inputs["tmlp_b1"], np.float32).reshape(128, 1),
        "tmlp_w2": tmlp_w2,
        "freqs4": freqs4.reshape(half, 1),
        "t_in": np.asarray(inputs["t"], np.float32).reshape(1, 1),
        "dummy_in": dummy,
    }
    b2c = tmlp_b2.reshape(256, 1)
    common["tmlp_b2cols"] = np.concatenate([b2c[:128], b2c[128:]], axis=1)
    for kk in ("What0", "What1", "What2", "fin_w1", "fin_w2", "dummy_in"):
        common[kk] = ml_bf16(common[kk])

    order = chunks["A"] + chunks["B"]
    in_maps = []
    for k in range(N_CORES):
        idx_blocks = []
        for (hf, tlist) in order:
            for (ii, cc) in tlist:
                blk = (prep["coreA"][k][ii] if hf == "A" else prep["coreB"][k][ii])
                idx_blocks.append(wrap_idx(blk[0:cc * 128]))
        idx_all = np.concatenate(idx_blocks, axis=1)
        m = dict(common)
        m["x0T_shard"] = ml_bf16(np.ascontiguousarray(xs[k].T))      # [128, L]
        m["qYT_shard"] = ml_bf16(np.ascontiguousarray(qYs[k].T))     # [8, L]
        m["idx_all"] = np.ascontiguousarray(idx_all)
        in_maps.append(m)
    return in_maps


def ml_bf16(a):
    return np.asarray(a).astype(np.float16)


# ----------------------------------------------------------------------------
# bass program
# ----------------------------------------------------------------------------
def build_program(prep, chunks, cmax, reps=1, no_ag=False, no_gather=False,
                  queues=4):
    order = chunks["A"] + chunks["B"]
    IDXC = sum(cc * 8 for (_, tlist) in order for (_, cc) in tlist)

    nc = bacc.Bacc("TRN2", target_bir_lowering=False, debug=False,
                   enable_asserts=False, num_devices=N_CORES,
                   num_swdge_queues=queues)

    x0T = nc.dram_tensor("x0T_shard", [128, L], BF, kind="ExternalInput")
    qYT = nc.dram_tensor("qYT_shard", [NH, L], BF, kind="ExternalInput")
    idx_all = nc.dram_tensor("idx_all", [128, IDXC], I16, kind="ExternalInput")
    What = [nc.dram_tensor(f"What{l}", [136 if l == 0 else 264, TW], BF,
                           kind="ExternalInput") for l in range(3)]
    b_rep = nc.dram_tensor("b_rep", [3, 128, HC], F32, kind="ExternalInput")
    fin_w1 = nc.dram_tensor("fin_w1", [264, 528], BF, kind="ExternalInput")
    fin_b1rep = nc.dram_tensor("fin_b1rep", [128, 528], F32, kind="ExternalInput")
    fin_w2 = nc.dram_tensor("fin_w2", [528, NH], BF, kind="ExternalInput")
    fin_b2rep = nc.dram_tensor("fin_b2rep", [128, NH], F32, kind="ExternalInput")
    tw1 = nc.dram_tensor("tmlp_w1", [128, 128], F32, kind="ExternalInput")
    tb1c = nc.dram_tensor("tmlp_b1col", [128, 1], F32, kind="ExternalInput")
    tw2 = nc.dram_tensor("tmlp_w2", [128, HC], F32, kind="ExternalInput")
    tb2c = nc.dram_tensor("tmlp_b2cols", [128, 2], F32, kind="ExternalInput")
    freqs4 = nc.dram_tensor("freqs4", [64, 1], F32, kind="ExternalInput")
    t_in = nc.dram_tensor("t_in", [1, 1], F32, kind="ExternalInput")
    dummy_in = nc.dram_tensor("dummy_in", [8, TW], BF, kind="ExternalInput")

    out = nc.dram_tensor("out", [L, NH], F32, kind="ExternalOutput")

    AGA = [nc.dram_tensor(f"aga{l}", [LA, TW], BF, kind="Internal")
           for l in range(3)]
    AGB = [nc.dram_tensor(f"agb{l}", [LB, TW], BF, kind="Internal")
           for l in range(3)]
    T_A = [nc.dram_tensor(f"tableA{l}", [N_CORES * LA, TW], BF, kind="Internal",
                          addr_space="Shared") for l in range(3)]
    T_B = [nc.dram_tensor(f"tableB{l}", [N_CORES * LB, TW], BF, kind="Internal",
                          addr_space="Shared") for l in range(3)]

    with tile.TileContext(nc) as tc:
        import contextlib
        with contextlib.ExitStack() as ctx:
            ctx.enter_context(nc.allow_low_precision(reason="fp16 edge path"))
            consts = ctx.enter_context(tc.tile_pool(name="consts", bufs=1))
            sb = ctx.enter_context(tc.tile_pool(name="sb", bufs=3))
            ps = ctx.enter_context(tc.tile_pool(name="ps", bufs=2, space="PSUM"))
            ps1 = ctx.enter_context(tc.tile_pool(name="ps1", bufs=1, space="PSUM"))
            gp = ctx.enter_context(tc.tile_pool(name="gp", bufs=3))

            ident = consts.tile([128, 128], F32)
            make_identity(nc, ident[:])
            identb = consts.tile([128, 128], BF, tag="identb")
            nc.vector.tensor_copy(out=identb[:], in_=ident[:])

            # dummy rows into AGIN tails (AG'd with each half)
            dt_ = consts.tile([8, TW], BF, tag="dummyt")
            nc.sync.dma_start(out=dt_[:], in_=dummy_in[:])
            for l in range(3):
                nc.sync.dma_start(out=AGA[l][LA - 8:LA, :], in_=dt_[:])
                nc.sync.dma_start(out=AGB[l][LB - 8:LB, :], in_=dt_[:])

            # ---- temb (f32) -> tb[l] tiles [128, 256] f32
            tcol = consts.tile([64, 1], F32, tag="tcol")
            nc.sync.dma_start(out=tcol[0:1, :], in_=t_in[:])
            nc.gpsimd.partition_broadcast(out_ap=tcol[:], in_ap=tcol[0:1, :])
            fq = consts.tile([64, 1], F32, tag="fq")
            nc.sync.dma_start(out=fq[:], in_=freqs4[:])
            xs_ = consts.tile([64, 1], F32, tag="xs")
            nc.vector.tensor_scalar_mul(xs_[:], tcol[:], 4.0)
            ang = consts.tile([64, 1], F32, tag="ang")
            nc.vector.tensor_tensor(out=ang[:], in0=xs_[:], in1=fq[:], op=OP.mult)
            TWO_PI = 2 * math.pi
            c1 = float(np.float32(TWO_PI))
            c2 = float(np.float32(TWO_PI - c1))
            c3 = float(TWO_PI - c1 - float(np.float32(TWO_PI - c1)))
            yk = consts.tile([64, 1], F32, tag="yk")
            nc.vector.tensor_scalar_mul(yk[:], ang[:], 1.0 / TWO_PI)
            ki = consts.tile([64, 1], mybir.dt.int32, tag="ki")
            nc.vector.tensor_copy(out=ki[:], in_=yk[:])
            kk_t = consts.tile([64, 1], F32, tag="kk_t")
            nc.vector.tensor_copy(out=kk_t[:], in_=ki[:])
            red = consts.tile([64, 1], F32, tag="red")
            nc.vector.cody_waite_cascade(out=red[:], x=ang[:], k=kk_t[:],
                                         c1=c1, c2=c2, c3=c3)
            rs = consts.tile([64, 1], F32, tag="rs")
            rc = consts.tile([64, 1], F32, tag="rc")
            nc.vector.add_range_wrap(out=rs[:], in_=red[:], shift=0.0,
                                     bound=math.pi, period=TWO_PI)
            nc.vector.add_range_wrap(out=rc[:], in_=red[:], shift=math.pi / 2,
                                     bound=math.pi, period=TWO_PI)
            sc = consts.tile([128, 1], F32, tag="sc")
            sc2 = consts.tile([64, 1], F32, tag="sc2")
            nc.scalar.activation(sc[0:64, :], rs[:], AF.Sin)
            nc.scalar.activation(sc2[:], rc[:], AF.Sin)
            nc.sync.dma_start(out=sc[64:128, :], in_=sc2[:])

            def elu_(xap, tmp_pool, shape, tag, dtype=BF):
                e = tmp_pool.tile(shape, dtype, tag=tag + "_e")
                r = tmp_pool.tile(shape, dtype, tag=tag + "_r")
                nc.scalar.activation(e[:], xap, AF.Exp)
                nc.vector.tensor_scalar(out=e[:], in0=e[:], scalar1=-1.0,
                                        scalar2=0.0, op0=OP.add, op1=OP.min)
                nc.scalar.activation(r[:], xap, AF.Relu)
                nc.vector.tensor_tensor(out=xap, in0=e[:], in1=r[:], op=OP.add)

            tw1_s = consts.tile([128, 128], F32, tag="tw1")
            nc.sync.dma_start(out=tw1_s[:], in_=tw1[:])
            tw2_s = consts.tile([128, HC], F32, tag="tw2")
            nc.sync.dma_start(out=tw2_s[:], in_=tw2[:])
            e1p = ps1.tile([128, 1], F32, tag="tembp")
            nc.tensor.matmul(out=e1p[:], lhsT=tw1_s[:], rhs=sc[:], start=True, stop=True)
            b1c = consts.tile([128, 1], F32, tag="tb1c")
            nc.sync.dma_start(out=b1c[:], in_=tb1c[:])
            e1 = consts.tile([128, 1], F32, tag="e1")
            nc.vector.tensor_tensor(out=e1[:], in0=e1p[:], in1=b1c[:], op=OP.add)
            elu_(e1[:], consts, [128, 1], "elu_temb", dtype=F32)
            tcols_p = ps1.tile([128, 2], F32, tag="tembp")
            nc.tensor.matmul(out=tcols_p[:, 0:1], lhsT=tw2_s[:, 0:128], rhs=e1[:],
                             start=True, stop=True)
            nc.tensor.matmul(out=tcols_p[:, 1:2], lhsT=tw2_s[:, 128:256], rhs=e1[:],
                             start=True, stop=True)
            b2c = consts.tile([128, 2], F32, tag="tb2c")
            nc.sync.dma_start(out=b2c[:], in_=tb2c[:])
            tcols = consts.tile([128, 2], F32, tag="tcols")
            nc.vector.tensor_tensor(out=tcols[:], in0=tcols_p[:], in1=b2c[:], op=OP.add)
            trow_p = ps1.tile([2, 128], F32, tag="tembp")
            nc.tensor.transpose(out=trow_p[:], in_=tcols[:], identity=ident[:])
            trow2 = consts.tile([2, 128], F32, tag="trow2")
            nc.scalar.copy(out=trow2[:], in_=trow_p[:])
            trow = consts.tile([1, HC], F32, tag="trow")
            nc.sync.dma_start(out=trow[0:1, 0:128], in_=trow2[0:1, :])
            nc.sync.dma_start(out=trow[0:1, 128:256], in_=trow2[1:2, :])
            temb_rep = consts.tile([128, HC], F32, tag="temb_rep")
            nc.gpsimd.partition_broadcast(out_ap=temb_rep[:], in_ap=trow[:])
            tb = []
            for l in range(3):
                bl = consts.tile([128, HC], F32, tag=f"b_rep{l}")
                nc.sync.dma_start(out=bl[:], in_=b_rep[l])
                tbl = consts.tile([128, HC], F32, tag=f"tb{l}")
                nc.vector.tensor_tensor(out=tbl[:], in0=temb_rep[:], in1=bl[:], op=OP.add)
                tb.append(tbl)

            # ---- weights into SBUF (bf16)
            Wchunks = []
            for l in range(3):
                F = 136 if l == 0 else 264
                cks = []
                off = 0
                while off < F:
                    kk = min(128, F - off)
                    wt = consts.tile([kk, TW], BF, tag=f"W{l}_{off}")
                    nc.sync.dma_start(out=wt[:], in_=What[l][off:off + kk, :])
                    cks.append((wt, kk))
                    off += kk
                Wchunks.append(cks)
            fw1 = []
            off = 0
            while off < 264:
                kk = min(128, 264 - off)
                wt = consts.tile([kk, 528], BF, tag=f"fw1_{off}")
                nc.sync.dma_start(out=wt[:], in_=fin_w1[off:off + kk, :])
                fw1.append((wt, kk))
                off += kk
            fw2 = []
            off = 0
            while off < 528:
                kk = min(128, 528 - off)
                wt = consts.tile([kk, NH], BF, tag=f"fw2_{off}")
                nc.sync.dma_start(out=wt[:], in_=fin_w2[off:off + kk, :])
                fw2.append((wt, kk))
                off += kk
            fb1 = consts.tile([128, 528], F32, tag="fb1")
            nc.sync.dma_start(out=fb1[:], in_=fin_b1rep[:])
            fb2 = consts.tile([128, NH], F32, tag="fb2")
            nc.sync.dma_start(out=fb2[:], in_=fin_b2rep[:])

            # ---- preloaded per-core data
            idx_sb = consts.tile([128, IDXC], I16, tag="idx_sb")
            nc.sync.dma_start(out=idx_sb[:], in_=idx_all[:])
            x0T_sb = consts.tile([128, L], BF, tag="x0T_sb")
            nc.sync.dma_start(out=x0T_sb[:], in_=x0T[:])
            qYT_sb = consts.tile([NH, L], BF, tag="qYT_sb")
            nc.sync.dma_start(out=qYT_sb[:], in_=qYT[:])

            # alpha_dst capture, double-buffered by layer parity
            ad_bufs = []
            for adi in range(2):
                ad_b = consts.tile([128, NT, NH], BF, tag=f"ad{adi}")
                ad_bufs.append(ad_b)
            # A-source partial sums per dest tile
            partial_h = consts.tile([128, NT, HC], BF, tag="partial_h")
            partial_d = consts.tile([128, NT, NH], F32, tag="partial_d")

            # ---- dense helper: lhsT chunks (each [kk,128] bf16) -> AGIN row
            def dense_tile(hT_chunks, l, i):
                pT = ps.tile([128, TW], F32, tag="pT")
                ncks = len(Wchunks[l])
                for ci, ((wt, kk), (ht_ap, kk2)) in enumerate(zip(Wchunks[l], hT_chunks)):
                    assert kk == kk2, (kk, kk2)
                    nc.tensor.matmul(out=pT[:], lhsT=ht_ap, rhs=wt[:],
                                     start=(ci == 0), stop=(ci == ncks - 1))
                Trow_s = sb.tile([128, TW], BF, tag="Trow_s")
                nc.scalar.copy(out=Trow_s[:], in_=pT[:])
                # capture alpha_dst for own rows into SBUF
                nc.vector.tensor_copy(out=ad_bufs[l % 2][:, i, :],
                                      in_=Trow_s[:, HC + NH:HC + 2 * NH])
                if i < TA_T:
                    nc.sync.dma_start(out=AGA[l][i * 128:(i + 1) * 128, :],
                                      in_=Trow_s[:])
                else:
                    j = i - TA_T
                    nc.sync.dma_start(out=AGB[l][j * 128:(j + 1) * 128, :],
                                      in_=Trow_s[:])

            def fire_ag(l, half):
                if no_ag:
                    return
                if half == "A":
                    nc.gpsimd.collective_compute(
                        "AllGather", OP.bypass,
                        replica_groups=[list(range(N_CORES))],
                        ins=[AGA[l][:]], outs=[T_A[l][:]])
                else:
                    nc.gpsimd.collective_compute(
                        "AllGather", OP.bypass,
                        replica_groups=[list(range(N_CORES))],
                        ins=[AGB[l][:]], outs=[T_B[l][:]])

            # ---- layer 0 dense (A tiles, AG-A, B tiles, AG-B)
            def dense0():
                for i in range(NT):
                    sl = slice(i * 128, (i + 1) * 128)
                    dense_tile([(x0T_sb[:, sl], 128), (qYT_sb[:, sl], 8)], 0, i)
                    if i == TA_T - 1:
                        fire_ag(0, "A")
                fire_ag(0, "B")

            # idx column offset per (half, tile) subchunk, in order
            idx_off = {}
            off = 0
            GMAX = 0
            for (hf, tlist) in order:
                GMAX = max(GMAX, sum(cc for (_, cc) in tlist))
                for (ii, cc) in tlist:
                    idx_off[(hf, ii)] = off
                    off += cc * 8

            def do_group(l, hf, tlist):
                """Gather + weights + in-place weighted h + per-tile tree
                reduce. Returns (g_tile, {tile: (col_off, den_tile)})."""
                tbl = T_A[l][:] if hf == "A" else T_B[l][:]
                qn = do_group.qctr % queues
                do_group.qctr += 1
                ccsum = sum(cc for (_, cc) in tlist)
                idxc = idx_off[(hf, tlist[0][0])]
                g_t = gp.tile([128, GMAX, TW], BF, tag="g")
                if no_gather:
                    base = (tlist[0][0] * 128 * 7) % (N_CORES * LB - 128 * ccsum)
                    nc.sync.dma_start(
                        out=g_t[:, 0:ccsum, :],
                        in_=tbl[base:base + 128 * ccsum, :]
                            .rearrange("(j p) w -> p j w", p=128))
                else:
                    nc.gpsimd.dma_gather(
                        out_ap=g_t[:, 0:ccsum, :], in_ap=tbl,
                        idxs_ap=idx_sb[:, idxc:idxc + ccsum * 8],
                        num_idxs=128 * ccsum, num_idxs_reg=128 * ccsum,
                        elem_size=TW, single_packet=False, queue_num=qn)
                ad = ad_bufs[l % 2]
                lg = sb.tile([128, GMAX, NH], BF, tag="lg")
                o = 0
                for (ii, cc) in tlist:
                    nc.vector.tensor_tensor(
                        out=lg[:, o:o + cc, :],
                        in0=g_t[:, o:o + cc, HC:HC + NH],
                        in1=ad[:, ii, :].unsqueeze(1).broadcast_to([128, cc, NH]),
                        op=OP.add)
                    o += cc
                nc.vector.scalar_tensor_tensor(
                    out=lg[:, 0:ccsum, :], in0=lg[:, 0:ccsum, :], scalar=0.2,
                    in1=lg[:, 0:ccsum, :], op0=OP.mult, op1=OP.max)
                w_t = sb.tile([128, GMAX, NH], BF, tag="w_t")
                nc.scalar.activation(w_t[:, 0:ccsum, :], lg[:, 0:ccsum, :],
                                     AF.Exp)
                nc.vector.tensor_tensor(
                    out=g_t[:, 0:ccsum, 0:HC].rearrange(
                        "p j (c h) -> p j c h", h=NH),
                    in0=g_t[:, 0:ccsum, 0:HC].rearrange(
                        "p j (c h) -> p j c h", h=NH),
                    in1=w_t[:, 0:ccsum, :].unsqueeze(2)
                        .broadcast_to([128, ccsum, 32, NH]),
                    op=OP.mult)
                info = {}
                o = 0
                for (ii, cc) in tlist:
                    den = sb.tile([128, NH], F32, tag=f"den{hf}")
                    nc.vector.tensor_reduce(
                        out=den[:, :, None],
                        in_=w_t[:, o:o + cc, :].rearrange("p j h -> p h j"),
                        axis=mybir.AxisListType.X, op=OP.add)
                    k = cc
                    while k > 1:
                        h2 = k // 2
                        lo = g_t[:, o:o + h2, 0:HC]
                        hi = g_t[:, o + k - h2:o + k, 0:HC]
                        nc.vector.tensor_tensor(out=lo, in0=lo, in1=hi,
                                                op=OP.add)
                        k -= h2
                    info[ii] = (o, den)
                    o += cc
                return g_t, info
            do_group.qctr = 0

            def finalize(l, gB, oB, denB, i):
                den = sb.tile([128, NH], F32, tag="den")
                nc.vector.tensor_tensor(out=den[:], in0=partial_d[:, i, :],
                                        in1=denB[:], op=OP.add)
                rcp = sb.tile([128, NH], BF, tag="rcp")
                nc.vector.reciprocal(rcp[:], den[:])
                h_t = sb.tile([128, HC], BF, tag="h_t")
                nc.vector.tensor_tensor(out=h_t[:], in0=partial_h[:, i, :],
                                        in1=gB[:, oB, 0:HC], op=OP.add)
                nc.vector.tensor_tensor(
                    out=h_t[:].rearrange("p (c h) -> p c h", h=NH),
                    in0=h_t[:].rearrange("p (c h) -> p c h", h=NH),
                    in1=rcp[:].unsqueeze(1).broadcast_to([128, 32, NH]),
                    op=OP.mult)
                nc.vector.tensor_tensor(out=h_t[:], in0=h_t[:],
                                        in1=tb[l][:], op=OP.add)
                elu_(h_t[:], sb, [128, HC], "elu_h")
                hts = []
                for ci, off2 in enumerate((0, 128)):
                    pt = ps.tile([128, 128], BF, tag="ptr")
                    nc.tensor.transpose(out=pt[:], in_=h_t[:, off2:off2 + 128],
                                        identity=identb[:])
                    st = sb.tile([128, 128], BF, tag=f"hT{ci}")
                    nc.scalar.copy(out=st[:], in_=pt[:])
                    hts.append((st, 128))
                qsl = qYT_sb[:, i * 128:(i + 1) * 128]
                if l < 2:
                    dense_tile([(hts[0][0][:], 128), (hts[1][0][:], 128),
                                (qsl, 8)], l + 1, i)
                else:
                    u = sb.tile([128, 528], BF, tag="u")
                    lhs3 = [hts[0][0][:], hts[1][0][:], qsl]
                    kks = [128, 128, 8]
                    for half_i in range(2):
                        pm = ps1.tile([128, 264], F32, tag="pmlp")
                        for ci in range(3):
                            nc.tensor.matmul(
                                out=pm[:],
                                lhsT=lhs3[ci],
                                rhs=fw1[ci][0][:kks[ci],
                                               half_i * 264:(half_i + 1) * 264],
                                start=(ci == 0), stop=(ci == 2))
                        nc.vector.tensor_tensor(
                            out=u[:, half_i * 264:(half_i + 1) * 264],
                            in0=pm[:],
                            in1=fb1[:, half_i * 264:(half_i + 1) * 264],
                            op=OP.add)
                    elu_(u[:], sb, [128, 528], "elu_u")
                    po = ps1.tile([128, NH], F32, tag="po")
                    uTs = []
                    for ci in range(5):
                        off2 = ci * 128
                        kk = min(128, 528 - off2)
                        pt = ps.tile([128, 128], BF, tag="ptr")
                        nc.tensor.transpose(out=pt[:kk, :],
                                            in_=u[:, off2:off2 + kk],
                                            identity=identb[:])
                        st = sb.tile([128, 128], BF, tag=f"uT{ci}")
                        nc.scalar.copy(out=st[:kk, :], in_=pt[:kk, :])
                        uTs.append((st, kk))
                    for ci, (st, kk) in enumerate(uTs):
                        nc.tensor.matmul(out=po[:], lhsT=st[:kk, :],
                                         rhs=fw2[ci][0][:kk, :],
                                         start=(ci == 0), stop=(ci == 4))
                    o_t = sb.tile([128, NH], F32, tag="o_t")
                    nc.vector.tensor_tensor(out=o_t[:], in0=po[:], in1=fb2[:],
                                            op=OP.add)
                    nc.sync.dma_start(out=out[i * 128:(i + 1) * 128, :],
                                      in_=o_t[:])

            def edge_layer(l):
                # A pass: all dest tiles' A-source blocks -> partials
                for (hf, tlist) in chunks["A"]:
                    g_t, info = do_group(l, hf, tlist)
                    for ii, (o, den) in info.items():
                        nc.vector.tensor_copy(out=partial_h[:, ii, :],
                                              in_=g_t[:, o, 0:HC])
                        nc.vector.tensor_copy(out=partial_d[:, ii, :],
                                              in_=den[:])
                # B pass: B-source blocks + finalize (+ next-layer dense & AGs)
                for (hf, tlist) in chunks["B"]:
                    g_t, info = do_group(l, hf, tlist)
                    for ii, (o, den) in info.items():
                        finalize(l, g_t, o, den, ii)
                        if l < 2:
                            if ii == TA_T - 1:
                                fire_ag(l + 1, "A")
                            elif ii == NT - 1:
                                fire_ag(l + 1, "B")

            for rep in range(reps):
                dense0()
                for l in range(3):
                    edge_layer(l)

    nc.compile()
    return nc


def run(inputs, cmax=26, trace=False):
    from concourse.bass_utils import run_bass_kernel_spmd
    from concourse.bass_interp import get_hw_module
    adj = np.asarray(inputs["adj"])
    n = int(np.asarray(inputs["x"]).shape[0])
    prep = preprocess(adj, n)
    chunks = build_chunks(prep, cmax)
    in_maps = host_inputs(inputs, prep, chunks)
    nc = build_program(prep, chunks, cmax)
    nc.m = get_hw_module(nc.m)
    res = run_bass_kernel_spmd(nc, in_maps, core_ids=list(range(N_CORES)),
                               trace=trace)
    outs = [np.asarray(r["out"]) for r in res.results]
    y = np.zeros((n, NH), np.float32)
    node_core, node_tile, node_p = (prep["node_core"], prep["node_tile"],
                                    prep["node_p"])
    rows = node_tile * 128 + node_p
    for k in range(N_CORES):
        m = node_core == k
        y[m] = outs[k][rows[m]]
    return y, res


def kernel(**inputs) -> np.ndarray:
    y, _ = run(inputs)
    return y
